# revision 1
# baseline (speedup 1.0000x reference)
"""Species-routed grouped matmul for Trainium2 (Bass/Tile), 8-core SPMD.

Problem: out[n, m, q] = sum_d x[n, m, d] * W[species_idx[n], d, q]
  x [16384, 64, 128] f32, species_idx [16384] int, W [8, 128, 128] f32.

Strategy
--------
Host (control-plane only): group sample indices by species and pad each
species' list to a multiple of 64 samples (8 cores x 8 samples/supertile) by
cycling indices of the *same* species.  Every core then receives an identical
static schedule: a list of "supertiles" (8 samples = 512 rows x 128), each
with a single species, so the per-supertile weight operand is a compile-time
SBUF slice of a resident W bank.  The permutation is applied while building
the per-core input shards; the inverse scatter is applied to the gathered
outputs (duplicate pad indices rewrite identical values, so no masking is
needed).

Device (per core, identical SPMD program):
  DMA in  : supertile slab (256 KiB contiguous; 2 KiB per partition)
  PE      : transpose each [128,128] sub-tile (fp32, via identity)
  DVE     : copy transposed tile PSUM -> SBUF
  PE      : fp32 matmul, lhsT = x_tile^T (stationary), rhs = W[s] slice
  DVE/ACT : copy result PSUM -> SBUF staging (alternate engines)
  DMA out : supertile slab back to DRAM

The kernel is HBM-bound by design (~134 MB/core at ~360 GB/s); everything
else pipelines underneath via Tile pools.
"""

import sys

sys.path.insert(0, "/opt/trn_rl_repo")

import numpy as np

import concourse.bass as bass
import concourse.mybir as mybir
from concourse import tile

N_SAMPLES = 16384
N_COMP = 64
D_IN = 128
D_OUT = 128
N_SPECIES = 8
N_CORES = 8

SS = 8  # samples per supertile (uniform species within a supertile)
ROWS_PER_SUPER = SS * N_COMP  # 512
SUBTILES = ROWS_PER_SUPER // 128  # 4
F32 = mybir.dt.float32

_PATCH_DONE = False


def _install_ntff_hook_shim():
    """The image's ``antenv`` package lacks ``axon_hooks``; ``bass_utils``
    unconditionally imports it on the trace path instead of degrading.
    Provide the module and register the ctypes NTFF hook from the boot
    helper so ``trace=True`` yields real hardware profiles."""
    import types

    try:
        import antenv.axon_hooks  # noqa: F401

        return
    except ImportError:
        pass
    mod = types.ModuleType("antenv.axon_hooks")
    holder = [None]
    mod.set_axon_ntff_profile_hook = lambda h: holder.__setitem__(0, h)
    mod.get_axon_ntff_profile_hook = lambda: holder[0]
    sys.modules["antenv.axon_hooks"] = mod
    try:
        import antenv

        antenv.axon_hooks = mod
    except ImportError:
        pass
    try:
        from trn_agent_boot.trn_boot import _ntff_profile_via_ctypes

        mod.set_axon_ntff_profile_hook(
            _ntff_profile_via_ctypes("/opt/axon/libaxon_pjrt.so")
        )
    except Exception:
        pass


_install_ntff_hook_shim()


def _apply_tile_patch():
    """Work around a walrus codegen limit on this toolchain: instructions on
    the CTRL (NO_STRUCT) path accept at most one sync wait, but TileContext's
    tail Drain carries one wait per outstanding semaphore.  Spill the excess
    waits onto dedicated single-wait SP nops emitted between the drain and
    the end barrier (the barrier publishes completion, so this is
    semantically identical)."""
    global _PATCH_DONE
    if _PATCH_DONE:
        return
    _PATCH_DONE = True

    from bass_rust import SyncInfo
    from concourse.vector_clock import ScopedClock

    max_waits = 1

    orig_lower = tile.TileContext._lower_ordered_insts

    def _lower_ordered_insts(self, ordered):
        """Spill excess sem waits (beyond max_waits) from any scheduled
        instruction onto same-engine NOPs inserted immediately before it.
        Same-engine program order makes this semantically identical."""
        n_spilled = 0
        for bb_name, insts in ordered.items():
            out = []
            for inst in insts:
                si = inst.sync_info
                if si is not None and si.on_wait and len(si.on_wait) > max_waits:
                    waits = list(si.on_wait)
                    si.on_wait = waits[:max_waits]
                    extra = waits[max_waits:]
                    for i in range(0, len(extra), max_waits):
                        nop = mybir.InstNoOp(
                            name=self.nc.get_next_instruction_name(),
                            engine=inst.engine,
                            bass_nofuse=True,
                            sync_info=SyncInfo(
                                on_wait=extra[i : i + max_waits], on_update=[]
                            ),
                        )
                        out.append(nop)
                        n_spilled += 1
                out.append(inst)
            insts[:] = out
        if n_spilled:
            print(f"[tile_patch] spilled waits onto {n_spilled} nops")
        return orig_lower(self, ordered)

    tile.TileContext._lower_ordered_insts = _lower_ordered_insts

    def _drain_and_barrier(self, tick_clock, wait_clock):
        nc = self.nc
        drain_inst = nc.sync.drain()
        wait_clock.add_sem_waits(
            drain_inst.ins, ScopedClock({None: tick_clock.global_clock})
        )
        si = drain_inst.ins.sync_info
        waits = list(si.on_wait) if si is not None and si.on_wait else []
        if len(waits) > max_waits:
            si.on_wait = waits[:max_waits]
            extra = waits[max_waits:]
            for i in range(0, len(extra), max_waits):
                nop = nc.sync.nop(nofuse=True, hint="drain_wait_spill")
                nop.ins.sync_info = SyncInfo(
                    on_wait=extra[i : i + max_waits], on_update=[]
                )
        nc.all_engine_barrier()
        assert self.sems is not None
        popped = nc._tile_sem_poison_stack.pop()
        assert popped is self._sem_poison
        nc.clear_and_free_semaphores(list(self.sems.allocated().values()))
        nc.all_engine_barrier()

    tile.TileContext._drain_and_barrier = _drain_and_barrier


def _plan(species_idx):
    """Build per-core permutations and the shared supertile species schedule.

    Returns (perms, sched): perms is a list of N_CORES int arrays, each of
    length 8 * sum(q_k) (sample indices into the full x, including pad
    repeats); sched is the per-supertile species id list shared by all cores.
    """
    s = np.asarray(species_idx).astype(np.int64).ravel()
    assert s.shape[0] == N_SAMPLES
    # jnp.take clamps out-of-range indices; mirror that for safety.
    s = np.clip(s, 0, N_SPECIES - 1)
    perms = [[] for _ in range(N_CORES)]
    sched = []
    group = N_CORES * SS  # 64: one supertile row across all cores
    for k in range(N_SPECIES):
        idx = np.nonzero(s == k)[0]
        if idx.size == 0:
            continue
        q_k = -(-idx.size // group)  # supertiles per core for this species
        padded = np.resize(idx, group * q_k)  # cycles same-species indices
        per_core = padded.reshape(N_CORES, SS * q_k)
        for c in range(N_CORES):
            perms[c].append(per_core[c])
        sched.extend([k] * q_k)
    perms = [np.concatenate(p) for p in perms]
    n_super = len(sched)
    for p in perms:
        assert p.size == n_super * SS
    return perms, sched


def _build_program(sched):
    """Trace the SPMD Bass program for the given supertile species schedule."""
    _apply_tile_patch()
    n_super = len(sched)
    rows = n_super * ROWS_PER_SUPER

    nc = bass.Bass()
    x = nc.declare_dram_parameter("x", [rows, D_IN], F32, isOutput=False)
    w = nc.declare_dram_parameter(
        "w", [N_SPECIES, D_IN, D_OUT], F32, isOutput=False
    )
    ident = nc.declare_dram_parameter("ident", [128, 128], F32, isOutput=False)
    y = nc.declare_dram_parameter("y", [rows, D_OUT], F32, isOutput=True)

    with tile.TileContext(nc) as tc:
        with (
            tc.tile_pool(name="wbank", bufs=1) as wpool,
            tc.tile_pool(name="ident", bufs=1) as ipool,
            tc.tile_pool(name="xin", bufs=10) as in_pool,
            tc.tile_pool(name="xt", bufs=8) as xt_pool,
            tc.tile_pool(name="yout", bufs=8) as out_pool,
            tc.tile_pool(name="pst", bufs=4, space="PSUM") as psum_t,
            tc.tile_pool(name="pso", bufs=4, space="PSUM") as psum_o,
        ):
            w_sb = wpool.tile([128, N_SPECIES * D_OUT], F32)
            nc.gpsimd.dma_start(
                out=w_sb[:].rearrange("d (s q) -> d s q", s=N_SPECIES),
                in_=w.rearrange("s d q -> d s q"),
            )
            id_sb = ipool.tile([128, 128], F32)
            nc.gpsimd.dma_start(out=id_sb[:], in_=ident[:])

            for u in range(n_super):
                sp = sched[u]
                r0 = u * ROWS_PER_SUPER
                w_slice = w_sb[:, sp * D_OUT : (sp + 1) * D_OUT]
                xin = in_pool.tile([128, ROWS_PER_SUPER], F32, tag="xin")
                nc.sync.dma_start(
                    out=xin[:],
                    in_=x[r0 : r0 + ROWS_PER_SUPER, :].rearrange(
                        "(p t) d -> p (t d)", p=128
                    ),
                )
                yout = out_pool.tile([128, ROWS_PER_SUPER], F32, tag="yout")
                # Pairs of 128-row sub-tiles share one single-bank [128,256]
                # PSUM tile, so PSUM->SBUF copies run at 256 wide (half the
                # per-op overhead).  Emit both transpose pairs (and their
                # copies) ahead of the matmuls so the PE always has ready
                # transpose work while a copy is in flight.
                xts = []
                for h in range(SUBTILES // 2):
                    pt = psum_t.tile([128, 256], F32, tag="pst")
                    xt = xt_pool.tile([128, 256], F32, tag="xt")
                    for j in range(2):
                        k = 2 * h + j
                        nc.tensor.transpose(
                            pt[:, j * 128 : (j + 1) * 128],
                            xin[:, k * 128 : (k + 1) * 128],
                            id_sb[:],
                        )
                    nc.vector.tensor_copy(xt[:], pt[:])
                    xts.append(xt)
                for h in range(SUBTILES // 2):
                    xt = xts[h]
                    po = psum_o.tile([128, 256], F32, tag="pso")
                    for j in range(2):
                        nc.tensor.matmul(
                            po[:, j * 128 : (j + 1) * 128],
                            xt[:, j * 128 : (j + 1) * 128],
                            w_slice,
                            start=True,
                            stop=True,
                        )
                    dst = yout[:, h * 256 : (h + 1) * 256]
                    if h % 2 == 0:
                        nc.vector.tensor_copy(dst, po[:])
                    else:
                        nc.scalar.copy(dst, po[:])
                nc.scalar.dma_start(
                    out=y[r0 : r0 + ROWS_PER_SUPER, :].rearrange(
                        "(p t) d -> p (t d)", p=128
                    ),
                    in_=yout[:],
                )
    return nc


def _run(x, species_idx, W, trace=False):
    from concourse.bass_utils import run_bass_kernel_spmd

    x = np.ascontiguousarray(np.asarray(x), dtype=np.float32)
    W = np.ascontiguousarray(np.asarray(W), dtype=np.float32)
    assert x.shape == (N_SAMPLES, N_COMP, D_IN)
    assert W.shape == (N_SPECIES, D_IN, D_OUT)

    perms, sched = _plan(species_idx)
    nc = _build_program(sched)

    ident = np.eye(128, dtype=np.float32)
    in_maps = []
    for c in range(N_CORES):
        xc = x[perms[c]].reshape(-1, D_IN)
        in_maps.append({"x": xc, "w": W, "ident": ident})

    res = run_bass_kernel_spmd(nc, in_maps, list(range(N_CORES)), trace=trace)

    out = np.empty((N_SAMPLES, N_COMP, D_OUT), dtype=np.float32)
    for c in range(N_CORES):
        yc = res.results[c]["y"].reshape(-1, N_COMP, D_OUT)
        out[perms[c]] = yc
    return out, res


def kernel(**inputs):
    out, _ = _run(inputs["x"], inputs["species_idx"], inputs["W"], trace=False)
    return out


def kernel_profiled(**inputs):
    return _run(inputs["x"], inputs["species_idx"], inputs["W"], trace=True)



# revision 2
# speedup vs baseline: 1.8279x; 1.8279x over previous
"""Species-routed grouped matmul for Trainium2 (Bass/Tile), 8-core SPMD.

Problem: out[n, m, q] = sum_d x[n, m, d] * W[species_idx[n], d, q]
  x [16384, 64, 128] f32, species_idx [16384] int, W [8, 128, 128] f32.

Strategy
--------
The kernel is HBM-bound, so halve the traffic: stage x to device DRAM as
bf16, pre-transposed on host into per-supertile [d=128, rows=512] slabs,
and write bf16 y^T back (rel-err budget is 2e-2; bf16 in/out costs ~2e-3).
The transposed staging also eliminates all PE transposes: with W[s] as the
stationary operand, one 512-wide bf16 matmul per supertile computes
y^T[q, rows] = W[s]^T x^T directly.

Host (control-plane only): group sample indices by species, pad each
species' list to a multiple of 64 samples (8 cores x 8 samples/supertile)
by cycling same-species indices, then pad the shared schedule to a multiple
of CHUNK supertiles.  Every core runs an identical static schedule; the
per-supertile weight operand is a compile-time SBUF slice of a resident W
bank.  The permutation is applied while building the per-core bf16 shards;
the inverse scatter + upcast is applied to the gathered outputs (duplicate
pad indices rewrite identical values).

Device (per core, identical SPMD program), per chunk of 4 supertiles:
  DMA in  : 512 KiB slab [128, 2048] bf16 (4 KiB per partition, sync/SP)
  PE      : 4x bf16 matmul, lhsT = W[s] (stationary), rhs = x^T slice
  DVE/ACT : copy+cast PSUM fp32 -> SBUF bf16 (alternating engines)
  DMA out : 512 KiB slab back to DRAM (scalar/Act)
"""

import sys

sys.path.insert(0, "/opt/trn_rl_repo")

import ml_dtypes
import numpy as np

import concourse.bass as bass
import concourse.mybir as mybir
from concourse import tile

N_SAMPLES = 16384
N_COMP = 64
D_IN = 128
D_OUT = 128
N_SPECIES = 8
N_CORES = 8

SS = 8  # samples per supertile (uniform species within a supertile)
ROWS_PER_SUPER = SS * N_COMP  # 512
CHUNK = 4  # supertiles per DMA slab
ROWS_PER_CHUNK = CHUNK * ROWS_PER_SUPER  # 2048
F32 = mybir.dt.float32
BF16 = mybir.dt.bfloat16
NP_BF16 = np.dtype(ml_dtypes.bfloat16)

_PATCH_DONE = False


def _install_ntff_hook_shim():
    """The image's ``antenv`` package lacks ``axon_hooks``; ``bass_utils``
    unconditionally imports it on the trace path instead of degrading.
    Provide the module and register the ctypes NTFF hook from the boot
    helper so ``trace=True`` yields real hardware profiles."""
    import types

    try:
        import antenv.axon_hooks  # noqa: F401

        return
    except ImportError:
        pass
    mod = types.ModuleType("antenv.axon_hooks")
    holder = [None]
    mod.set_axon_ntff_profile_hook = lambda h: holder.__setitem__(0, h)
    mod.get_axon_ntff_profile_hook = lambda: holder[0]
    sys.modules["antenv.axon_hooks"] = mod
    try:
        import antenv

        antenv.axon_hooks = mod
    except ImportError:
        pass
    try:
        from trn_agent_boot.trn_boot import _ntff_profile_via_ctypes

        mod.set_axon_ntff_profile_hook(
            _ntff_profile_via_ctypes("/opt/axon/libaxon_pjrt.so")
        )
    except Exception:
        pass


_install_ntff_hook_shim()


def _apply_tile_patch():
    """Work around a walrus codegen limit on this toolchain: instructions on
    the CTRL (NO_STRUCT) path accept at most one sync wait, but TileContext's
    tail Drain carries one wait per outstanding semaphore.  Spill the excess
    waits onto dedicated single-wait SP nops emitted between the drain and
    the end barrier (the barrier publishes completion, so this is
    semantically identical)."""
    global _PATCH_DONE
    if _PATCH_DONE:
        return
    _PATCH_DONE = True

    from bass_rust import SyncInfo
    from concourse.vector_clock import ScopedClock

    max_waits = 1

    orig_lower = tile.TileContext._lower_ordered_insts

    def _lower_ordered_insts(self, ordered):
        """Spill excess sem waits (beyond max_waits) from any scheduled
        instruction onto same-engine NOPs inserted immediately before it.
        Same-engine program order makes this semantically identical."""
        n_spilled = 0
        for bb_name, insts in ordered.items():
            out = []
            for inst in insts:
                si = inst.sync_info
                if si is not None and si.on_wait and len(si.on_wait) > max_waits:
                    waits = list(si.on_wait)
                    si.on_wait = waits[:max_waits]
                    extra = waits[max_waits:]
                    for i in range(0, len(extra), max_waits):
                        nop = mybir.InstNoOp(
                            name=self.nc.get_next_instruction_name(),
                            engine=inst.engine,
                            bass_nofuse=True,
                            sync_info=SyncInfo(
                                on_wait=extra[i : i + max_waits], on_update=[]
                            ),
                        )
                        out.append(nop)
                        n_spilled += 1
                out.append(inst)
            insts[:] = out
        if n_spilled:
            print(f"[tile_patch] spilled waits onto {n_spilled} nops")
        return orig_lower(self, ordered)

    tile.TileContext._lower_ordered_insts = _lower_ordered_insts

    def _drain_and_barrier(self, tick_clock, wait_clock):
        nc = self.nc
        drain_inst = nc.sync.drain()
        wait_clock.add_sem_waits(
            drain_inst.ins, ScopedClock({None: tick_clock.global_clock})
        )
        si = drain_inst.ins.sync_info
        waits = list(si.on_wait) if si is not None and si.on_wait else []
        if len(waits) > max_waits:
            si.on_wait = waits[:max_waits]
            extra = waits[max_waits:]
            for i in range(0, len(extra), max_waits):
                nop = nc.sync.nop(nofuse=True, hint="drain_wait_spill")
                nop.ins.sync_info = SyncInfo(
                    on_wait=extra[i : i + max_waits], on_update=[]
                )
        nc.all_engine_barrier()
        assert self.sems is not None
        popped = nc._tile_sem_poison_stack.pop()
        assert popped is self._sem_poison
        nc.clear_and_free_semaphores(list(self.sems.allocated().values()))
        nc.all_engine_barrier()

    tile.TileContext._drain_and_barrier = _drain_and_barrier


def _plan(species_idx):
    """Build per-core permutations and the shared supertile species schedule.

    Returns (perms, sched): perms is a list of N_CORES int arrays, each of
    length SS * len(sched) (sample indices into the full x, including pad
    repeats); sched is the per-supertile species id list shared by all
    cores, padded to a multiple of CHUNK.
    """
    s = np.asarray(species_idx).astype(np.int64).ravel()
    assert s.shape[0] == N_SAMPLES
    # jnp.take clamps out-of-range indices; mirror that for safety.
    s = np.clip(s, 0, N_SPECIES - 1)
    perms = [[] for _ in range(N_CORES)]
    sched = []
    group = N_CORES * SS  # 64: one supertile row across all cores
    for k in range(N_SPECIES):
        idx = np.nonzero(s == k)[0]
        if idx.size == 0:
            continue
        q_k = -(-idx.size // group)  # supertiles per core for this species
        padded = np.resize(idx, group * q_k)  # cycles same-species indices
        per_core = padded.reshape(N_CORES, SS * q_k)
        for c in range(N_CORES):
            perms[c].append(per_core[c])
        sched.extend([k] * q_k)
    # Pad the schedule to a CHUNK multiple by replaying the last supertile.
    perms = [np.concatenate(p) for p in perms]
    while len(sched) % CHUNK:
        sched.append(sched[-1])
        perms = [np.concatenate([p, p[-SS:]]) for p in perms]
    n_super = len(sched)
    for p in perms:
        assert p.size == n_super * SS
    return perms, sched


def _build_program(sched):
    """Trace the SPMD Bass program for the given supertile species schedule."""
    _apply_tile_patch()
    n_super = len(sched)
    assert n_super % CHUNK == 0
    n_chunks = n_super // CHUNK

    nc = bass.Bass()
    x = nc.declare_dram_parameter(
        "x", [n_chunks * 128, ROWS_PER_CHUNK], BF16, isOutput=False
    )
    w = nc.declare_dram_parameter(
        "w", [D_IN, N_SPECIES * D_OUT], BF16, isOutput=False
    )
    y = nc.declare_dram_parameter(
        "y", [n_chunks * 128, ROWS_PER_CHUNK], BF16, isOutput=True
    )

    with tile.TileContext(nc) as tc:
        with (
            tc.tile_pool(name="wbank", bufs=1) as wpool,
            tc.tile_pool(name="xin", bufs=6) as in_pool,
            tc.tile_pool(name="yout", bufs=6) as out_pool,
            tc.tile_pool(name="pso", bufs=8, space="PSUM") as psum_o,
        ):
            w_sb = wpool.tile([128, N_SPECIES * D_OUT], BF16)
            nc.gpsimd.dma_start(out=w_sb[:], in_=w[:])

            for c in range(n_chunks):
                r0 = c * 128
                xin = in_pool.tile([128, ROWS_PER_CHUNK], BF16, tag="xin")
                nc.sync.dma_start(
                    out=xin[:], in_=x[r0 : r0 + 128, :]
                )
                yout = out_pool.tile([128, ROWS_PER_CHUNK], BF16, tag="yout")
                for u in range(CHUNK):
                    sp = sched[c * CHUNK + u]
                    po = psum_o.tile([128, ROWS_PER_SUPER], F32, tag="pso")
                    nc.tensor.matmul(
                        po[:],
                        w_sb[:, sp * D_OUT : (sp + 1) * D_OUT],
                        xin[:, u * ROWS_PER_SUPER : (u + 1) * ROWS_PER_SUPER],
                        start=True,
                        stop=True,
                    )
                    dst = yout[:, u * ROWS_PER_SUPER : (u + 1) * ROWS_PER_SUPER]
                    if u % 2 == 0:
                        nc.vector.tensor_copy(dst, po[:])
                    else:
                        nc.scalar.copy(dst, po[:])
                nc.scalar.dma_start(out=y[r0 : r0 + 128, :], in_=yout[:])
    return nc


def _run(x, species_idx, W, trace=False):
    from concourse.bass_utils import run_bass_kernel_spmd

    x = np.ascontiguousarray(np.asarray(x), dtype=np.float32)
    W = np.ascontiguousarray(np.asarray(W), dtype=np.float32)
    assert x.shape == (N_SAMPLES, N_COMP, D_IN)
    assert W.shape == (N_SPECIES, D_IN, D_OUT)

    perms, sched = _plan(species_idx)
    nc = _build_program(sched)
    n_super = len(sched)
    n_chunks = n_super // CHUNK

    # bf16 staging (as uint16 for fast numpy reshuffles)
    xb = x.astype(NP_BF16).view(np.uint16)  # [N, M, D]
    wt = (
        W.astype(NP_BF16).transpose(1, 0, 2).reshape(D_IN, N_SPECIES * D_OUT)
    )
    wt = np.ascontiguousarray(wt)

    in_maps = []
    for c in range(N_CORES):
        xg = xb[perms[c]].reshape(n_chunks, ROWS_PER_CHUNK, D_IN)
        xg = np.ascontiguousarray(xg.transpose(0, 2, 1))  # [nc, d, rows]
        in_maps.append(
            {
                "x": xg.reshape(n_chunks * 128, ROWS_PER_CHUNK).view(NP_BF16),
                "w": wt,
            }
        )

    res = run_bass_kernel_spmd(nc, in_maps, list(range(N_CORES)), trace=trace)

    outb = np.empty((N_SAMPLES, N_COMP, D_OUT), dtype=np.uint16)
    for c in range(N_CORES):
        yc = np.asarray(res.results[c]["y"]).view(np.uint16)
        yc = yc.reshape(n_chunks, 128, ROWS_PER_CHUNK).transpose(0, 2, 1)
        outb[perms[c]] = yc.reshape(-1, N_COMP, D_OUT)
    return outb.view(NP_BF16).astype(np.float32), res


def kernel(**inputs):
    out, _ = _run(inputs["x"], inputs["species_idx"], inputs["W"], trace=False)
    return out


def kernel_profiled(**inputs):
    return _run(inputs["x"], inputs["species_idx"], inputs["W"], trace=True)


# revision 4
# speedup vs baseline: 2.1353x; 1.1682x over previous
"""Species-routed grouped matmul for Trainium2 (Bass/Tile), 8-core SPMD.

Problem: out[n, m, q] = sum_d x[n, m, d] * W[species_idx[n], d, q]
  x [16384, 64, 128] f32, species_idx [16384] int, W [8, 128, 128] f32.

Strategy
--------
The kernel is HBM-bound, so halve the traffic: stage x to device DRAM as
bf16, pre-transposed on host into per-supertile [d=128, rows=512] slabs,
and write bf16 y^T back (rel-err budget is 2e-2; bf16 in/out costs ~2e-3).
The transposed staging also eliminates all PE transposes: with W[s] as the
stationary operand, one 512-wide bf16 matmul per supertile computes
y^T[q, rows] = W[s]^T x^T directly.

Host (control-plane only): group sample indices by species, pad each
species' list to a multiple of 64 samples (8 cores x 8 samples/supertile)
by cycling same-species indices, then pad the shared schedule to a multiple
of CHUNK supertiles.  Every core runs an identical static schedule; the
per-supertile weight operand is a compile-time SBUF slice of a resident W
bank.  The permutation is applied while building the per-core bf16 shards;
the inverse scatter + upcast is applied to the gathered outputs (duplicate
pad indices rewrite identical values).

Device (per core, identical SPMD program), per chunk of 4 supertiles:
  DMA in  : 512 KiB slab [128, 2048] bf16 (4 KiB per partition, sync/SP)
  PE      : 4x bf16 matmul, lhsT = W[s] (stationary), rhs = x^T slice
  DVE/ACT : copy+cast PSUM fp32 -> SBUF bf16 (alternating engines)
  DMA out : 512 KiB slab back to DRAM (scalar/Act)
"""

import sys

sys.path.insert(0, "/opt/trn_rl_repo")

import ml_dtypes
import numpy as np

import concourse.bass as bass
import concourse.mybir as mybir
from concourse import tile

N_SAMPLES = 16384
N_COMP = 64
D_IN = 128
D_OUT = 128
N_SPECIES = 8
N_CORES = 8

SS = 8  # samples per supertile (uniform species within a supertile)
ROWS_PER_SUPER = SS * N_COMP  # 512
CHUNK = 2  # supertiles per DMA slab
ROWS_PER_CHUNK = CHUNK * ROWS_PER_SUPER  # 1024
F32 = mybir.dt.float32
BF16 = mybir.dt.bfloat16
NP_BF16 = np.dtype(ml_dtypes.bfloat16)

_PATCH_DONE = False


def _install_ntff_hook_shim():
    """The image's ``antenv`` package lacks ``axon_hooks``; ``bass_utils``
    unconditionally imports it on the trace path instead of degrading.
    Provide the module and register the ctypes NTFF hook from the boot
    helper so ``trace=True`` yields real hardware profiles."""
    import types

    try:
        import antenv.axon_hooks  # noqa: F401

        return
    except ImportError:
        pass
    mod = types.ModuleType("antenv.axon_hooks")
    holder = [None]
    mod.set_axon_ntff_profile_hook = lambda h: holder.__setitem__(0, h)
    mod.get_axon_ntff_profile_hook = lambda: holder[0]
    sys.modules["antenv.axon_hooks"] = mod
    try:
        import antenv

        antenv.axon_hooks = mod
    except ImportError:
        pass
    try:
        from trn_agent_boot.trn_boot import _ntff_profile_via_ctypes

        mod.set_axon_ntff_profile_hook(
            _ntff_profile_via_ctypes("/opt/axon/libaxon_pjrt.so")
        )
    except Exception:
        pass


_install_ntff_hook_shim()


def _apply_tile_patch():
    """Work around a walrus codegen limit on this toolchain: instructions on
    the CTRL (NO_STRUCT) path accept at most one sync wait, but TileContext's
    tail Drain carries one wait per outstanding semaphore.  Spill the excess
    waits onto dedicated single-wait SP nops emitted between the drain and
    the end barrier (the barrier publishes completion, so this is
    semantically identical)."""
    global _PATCH_DONE
    if _PATCH_DONE:
        return
    _PATCH_DONE = True

    from bass_rust import SyncInfo
    from concourse.vector_clock import ScopedClock

    max_waits = 1

    orig_lower = tile.TileContext._lower_ordered_insts

    def _lower_ordered_insts(self, ordered):
        """Spill excess sem waits (beyond max_waits) from any scheduled
        instruction onto same-engine NOPs inserted immediately before it.
        Same-engine program order makes this semantically identical."""
        n_spilled = 0
        for bb_name, insts in ordered.items():
            out = []
            for inst in insts:
                si = inst.sync_info
                if si is not None and si.on_wait and len(si.on_wait) > max_waits:
                    waits = list(si.on_wait)
                    si.on_wait = waits[:max_waits]
                    extra = waits[max_waits:]
                    for i in range(0, len(extra), max_waits):
                        nop = mybir.InstNoOp(
                            name=self.nc.get_next_instruction_name(),
                            engine=inst.engine,
                            bass_nofuse=True,
                            sync_info=SyncInfo(
                                on_wait=extra[i : i + max_waits], on_update=[]
                            ),
                        )
                        out.append(nop)
                        n_spilled += 1
                out.append(inst)
            insts[:] = out
        if n_spilled:
            print(f"[tile_patch] spilled waits onto {n_spilled} nops")
        return orig_lower(self, ordered)

    tile.TileContext._lower_ordered_insts = _lower_ordered_insts

    def _drain_and_barrier(self, tick_clock, wait_clock):
        nc = self.nc
        drain_inst = nc.sync.drain()
        wait_clock.add_sem_waits(
            drain_inst.ins, ScopedClock({None: tick_clock.global_clock})
        )
        si = drain_inst.ins.sync_info
        waits = list(si.on_wait) if si is not None and si.on_wait else []
        if len(waits) > max_waits:
            si.on_wait = waits[:max_waits]
            extra = waits[max_waits:]
            for i in range(0, len(extra), max_waits):
                nop = nc.sync.nop(nofuse=True, hint="drain_wait_spill")
                nop.ins.sync_info = SyncInfo(
                    on_wait=extra[i : i + max_waits], on_update=[]
                )
        nc.all_engine_barrier()
        assert self.sems is not None
        popped = nc._tile_sem_poison_stack.pop()
        assert popped is self._sem_poison
        nc.clear_and_free_semaphores(list(self.sems.allocated().values()))
        nc.all_engine_barrier()

    tile.TileContext._drain_and_barrier = _drain_and_barrier


def _plan(species_idx):
    """Build per-core permutations and the shared supertile species schedule.

    Returns (perms, sched): perms is a list of N_CORES int arrays, each of
    length SS * len(sched) (sample indices into the full x, including pad
    repeats); sched is the per-supertile species id list shared by all
    cores, padded to a multiple of CHUNK.
    """
    s = np.asarray(species_idx).astype(np.int64).ravel()
    assert s.shape[0] == N_SAMPLES
    # jnp.take clamps out-of-range indices; mirror that for safety.
    s = np.clip(s, 0, N_SPECIES - 1)
    perms = [[] for _ in range(N_CORES)]
    sched = []
    group = N_CORES * SS  # 64: one supertile row across all cores
    for k in range(N_SPECIES):
        idx = np.nonzero(s == k)[0]
        if idx.size == 0:
            continue
        q_k = -(-idx.size // group)  # supertiles per core for this species
        padded = np.resize(idx, group * q_k)  # cycles same-species indices
        per_core = padded.reshape(N_CORES, SS * q_k)
        for c in range(N_CORES):
            perms[c].append(per_core[c])
        sched.extend([k] * q_k)
    # Pad the schedule to a CHUNK multiple by replaying the last supertile.
    perms = [np.concatenate(p) for p in perms]
    while len(sched) % CHUNK:
        sched.append(sched[-1])
        perms = [np.concatenate([p, p[-SS:]]) for p in perms]
    n_super = len(sched)
    for p in perms:
        assert p.size == n_super * SS
    return perms, sched


def _build_program(sched):
    """Trace the SPMD Bass program for the given supertile species schedule."""
    _apply_tile_patch()
    n_super = len(sched)
    assert n_super % CHUNK == 0
    n_chunks = n_super // CHUNK

    nc = bass.Bass()
    x = nc.declare_dram_parameter(
        "x", [n_chunks * 128, ROWS_PER_CHUNK], BF16, isOutput=False
    )
    w = nc.declare_dram_parameter(
        "w", [D_IN, N_SPECIES * D_OUT], BF16, isOutput=False
    )
    y = nc.declare_dram_parameter(
        "y", [n_chunks * 128, ROWS_PER_CHUNK], BF16, isOutput=True
    )

    with tile.TileContext(nc) as tc:
        with (
            tc.tile_pool(name="wbank", bufs=1) as wpool,
            tc.tile_pool(name="xin", bufs=8) as in_pool,
            tc.tile_pool(name="yout", bufs=8) as out_pool,
            tc.tile_pool(name="pso", bufs=8, space="PSUM") as psum_o,
        ):
            w_sb = wpool.tile([128, N_SPECIES * D_OUT], BF16)
            nc.sync.dma_start(out=w_sb[:], in_=w[:])

            for c in range(n_chunks):
                r0 = c * 128
                xin = in_pool.tile([128, ROWS_PER_CHUNK], BF16, tag="xin")
                nc.sync.dma_start(
                    out=xin[:], in_=x[r0 : r0 + 128, :]
                )
                yout = out_pool.tile([128, ROWS_PER_CHUNK], BF16, tag="yout")
                for u in range(CHUNK):
                    sp = sched[c * CHUNK + u]
                    po = psum_o.tile([128, ROWS_PER_SUPER], F32, tag="pso")
                    nc.tensor.matmul(
                        po[:],
                        w_sb[:, sp * D_OUT : (sp + 1) * D_OUT],
                        xin[:, u * ROWS_PER_SUPER : (u + 1) * ROWS_PER_SUPER],
                        start=True,
                        stop=True,
                    )
                    dst = yout[:, u * ROWS_PER_SUPER : (u + 1) * ROWS_PER_SUPER]
                    if u % 2 == 0:
                        nc.vector.tensor_copy(dst, po[:])
                    else:
                        nc.scalar.copy(dst, po[:])
                nc.scalar.dma_start(out=y[r0 : r0 + 128, :], in_=yout[:])
    return nc


def _run(x, species_idx, W, trace=False):
    from concourse.bass_utils import run_bass_kernel_spmd

    x = np.ascontiguousarray(np.asarray(x), dtype=np.float32)
    W = np.ascontiguousarray(np.asarray(W), dtype=np.float32)
    assert x.shape == (N_SAMPLES, N_COMP, D_IN)
    assert W.shape == (N_SPECIES, D_IN, D_OUT)

    perms, sched = _plan(species_idx)
    nc = _build_program(sched)
    n_super = len(sched)
    n_chunks = n_super // CHUNK

    # bf16 staging (as uint16 for fast numpy reshuffles)
    xb = x.astype(NP_BF16).view(np.uint16)  # [N, M, D]
    wt = (
        W.astype(NP_BF16).transpose(1, 0, 2).reshape(D_IN, N_SPECIES * D_OUT)
    )
    wt = np.ascontiguousarray(wt)

    in_maps = []
    for c in range(N_CORES):
        xg = xb[perms[c]].reshape(n_chunks, ROWS_PER_CHUNK, D_IN)
        xg = np.ascontiguousarray(xg.transpose(0, 2, 1))  # [nc, d, rows]
        in_maps.append(
            {
                "x": xg.reshape(n_chunks * 128, ROWS_PER_CHUNK).view(NP_BF16),
                "w": wt,
            }
        )

    res = run_bass_kernel_spmd(nc, in_maps, list(range(N_CORES)), trace=trace)

    outb = np.empty((N_SAMPLES, N_COMP, D_OUT), dtype=np.uint16)
    for c in range(N_CORES):
        yc = np.asarray(res.results[c]["y"]).view(np.uint16)
        yc = yc.reshape(n_chunks, 128, ROWS_PER_CHUNK).transpose(0, 2, 1)
        outb[perms[c]] = yc.reshape(-1, N_COMP, D_OUT)
    return outb.view(NP_BF16).astype(np.float32), res


def kernel(**inputs):
    out, _ = _run(inputs["x"], inputs["species_idx"], inputs["W"], trace=False)
    return out


def kernel_profiled(**inputs):
    return _run(inputs["x"], inputs["species_idx"], inputs["W"], trace=True)
